# revision 22
# baseline (speedup 1.0000x reference)
"""Trainium2 Bass kernel for nn_ContextRelation_Module (dense_transformer).

Data-parallel over batch: 8 batches -> 8 NeuronCores, one batch each.

Per-core program (B=1 slice); q-branch in fp8e4m3, attention/y in bf16,
PSUM fp32:
  x     [512, 16384]  (C_in, H*W)  fp8
  q1    = relu(W_q1' @ x + b1)            [256, 16384]   (BN scale folded into W)
  q2    = relu(W_q2' @ q1 + b2)           [256, 16384]
  k,v   = relu-stacks of the 19-token context  (preamble)
  Wv    = W_u' @ v                         [512, 19]      (preamble;  y-GEMM collapse:
                                            y = relu(W_u'@(v@attT) + bu) = relu(Wv@attT + bu))
  simT  = k^T @ q2 * (1/16)                [19, 16384]
  esim  = exp(simT)
  den   = ones[19,19]^T @ esim             [19, 16384]   (denominator, pre-broadcast)
  att   = esim * approx_recip(den)         [19, 16384]
  y     = relu(Wv^T.T @ att + bu)          [512, 16384]

Key design points:
  * y = (uW@v) @ attT: the [K,HW] ctx tensor and its 8 matmuls + 2 PSUM
    drains per tile collapse into 4 contraction-19 matmuls (26 -> 19
    N=512 matmuls per tile on the PE).
  * fp8 q-branch: the softmax logits are tiny (sim*1/16), so attention
    output is insensitive to q-path quantization (measured 2.9e-3
    absmax-rel, same as all-bf16).  x HBM traffic drops to 8 MB.
  * BN scale folded into weights host-side; every BN-ReLU is a single
    activation/tensor_scalar op.
  * softmax denominator broadcast merged into the ones-matmul
    (ones[19,19]); reciprocal via the ~5x faster approx custom DVE op.
  * pointwise split: Scalar (q relus, exp, 2 y-relus), Vector (recip,
    att mul, 2 y-relus), GpSimd (y DMA).
  * x DMAed in ramped chunks (1,1,2,8,... tiles) so the first q1 matmul
    is not gated on a large transfer; constant DMA issues are spread
    across the sync/scalar/gpsimd queues (each software-DGE dma_start
    costs ~0.7us of issuing-engine time).

The per-512-column tile pipeline is software-pipelined across 4 stages
(den | y | att | x-dma+projections+sim+exp); thin matmuls (den, y) are
interleaved between the fat q-groups, and the DVE recip is emitted after
the y-relus so no engine queue head-of-line blocks on a fresh value.
Measured on HW (NTFF profile): ~202 us device span per launch
(vs 316 us for the v1 fp32r kernel), PE-bound at ~300 ns per
LDWEIGHTS+MATMUL pair.  DoubleRow fp8 was tried and measured SLOWER
(~750 ns per K=256 matmul - the walrus codegen path streams the two
interleaved planes sequentially); fp8 here buys HBM traffic and weight-
load time, not matmul rate.
"""

import numpy as np

import concourse.bacc as bacc
import concourse.bass as bass
import concourse.mybir as mybir
import concourse.tile as tile
from concourse import bass_utils
from concourse.bass import ts
from concourse import bass_isa

AFT = mybir.ActivationFunctionType
ALU = mybir.AluOpType
F32 = mybir.dt.float32
BF = mybir.dt.bfloat16
F8 = mybir.dt.float8e4
BF_NP = mybir.dt.np(mybir.dt.bfloat16)
F8_NP = mybir.dt.np(mybir.dt.float8e4)

# problem dims (hardcoded per contract)
B = 8
C = 512            # input/output channels
K = 256            # key_channels
H = 128
W = 128
NCTX = 19          # context tokens
NPIX = H * W       # 16384 pixels per batch
CB = C // 128      # 4 partition blocks of C
KB = K // 128      # 2 partition blocks of K
EPS = 1e-5
SOFTMAX_SCALE = K ** -0.5   # 1/16

# tunables
TN = 512           # free-dim tile (one PSUM bank of fp32)
CHMAX = 8          # steady-state pixel tiles per x-DMA chunk


def _chunk_plan(nt):
    """Ramped chunk sizes so tile 0 is gated on a 1-tile transfer."""
    sizes = []
    for s in (1, 1, 2):
        if sum(sizes) >= nt:
            break
        sizes.append(min(s, nt - sum(sizes)))
    while sum(sizes) < nt:
        sizes.append(min(CHMAX, nt - sum(sizes)))
    starts = [sum(sizes[:i]) for i in range(len(sizes))]
    return sizes, starts


def _build(npix=NPIX, repeat=1):
    """Build + compile the per-core Bass module.

    repeat>1 re-runs the whole pixel loop (same input/output) for
    differential timing: t(R) - t(1) = (R-1) * T_kernel.
    """
    nt = npix // TN
    ch_sizes, ch_starts = _chunk_plan(nt)
    nch = len(ch_sizes)
    chunk_of = []
    for i, s in enumerate(ch_sizes):
        chunk_of += [i] * s
    nc = bacc.Bacc("TRN2", target_bir_lowering=False, debug=False)

    x_d = nc.dram_tensor("x", [C, npix], F8, kind="ExternalInput").ap()
    ct_d = nc.dram_tensor("ctxt", [C, NCTX], BF, kind="ExternalInput").ap()
    wq1_d = nc.dram_tensor("wq1", [C, K], F8, kind="ExternalInput").ap()
    wq2_d = nc.dram_tensor("wq2", [K, K], F8, kind="ExternalInput").ap()
    wk1_d = nc.dram_tensor("wk1", [C, K], BF, kind="ExternalInput").ap()
    wk2_d = nc.dram_tensor("wk2", [K, K], BF, kind="ExternalInput").ap()
    wv_d = nc.dram_tensor("wv", [C, K], BF, kind="ExternalInput").ap()
    wu_d = nc.dram_tensor("wu", [K, C], BF, kind="ExternalInput").ap()
    # biases packed into one tensor: [bq1|bq2|bk1|bk2|bv](2 each) + bu(4)
    ball_d = nc.dram_tensor("ball", [128, 14], F32, kind="ExternalInput").ap()
    y_d = nc.dram_tensor("y", [C, npix], BF, kind="ExternalOutput").ap()

    x_v = x_d.rearrange("(c p) n -> p c n", p=128)
    y_v = y_d.rearrange("(c p) n -> p c n", p=128)

    mm = nc.tensor.matmul

    with tile.TileContext(nc) as tc, nc.allow_low_precision(reason="fp8/bf16 operands"):
        with (
            tc.tile_pool(name="consts", bufs=1) as consts,
            tc.tile_pool(name="xin", bufs=3) as xin,
            tc.tile_pool(name="yout", bufs=2) as yout,
            tc.tile_pool(name="work", bufs=2) as work,
            tc.tile_pool(name="psQ", bufs=4, space="PSUM") as psQ,
            tc.tile_pool(name="psS", bufs=2, space="PSUM") as psS,
            tc.tile_pool(name="psY", bufs=2, space="PSUM") as psY,
        ):
            # ---- constants; DMA issues spread across engine queues so the
            # ~0.7us-per-issue descriptor cost does not serialize startup ----
            wk1_sb = consts.tile([128, CB, K], BF, name="wk1_sb")
            nc.sync.dma_start(out=wk1_sb, in_=wk1_d.rearrange("(c p) m -> p c m", p=128))
            ct_sb = consts.tile([128, CB, NCTX], BF, name="ct_sb")
            nc.scalar.dma_start(out=ct_sb, in_=ct_d.rearrange("(c p) m -> p c m", p=128))
            ball_sb = consts.tile([128, 14], F32, name="ball_sb")
            nc.gpsimd.dma_start(out=ball_sb, in_=ball_d)
            bias = {"bq1": ball_sb[:, 0:2], "bq2": ball_sb[:, 2:4],
                    "bk1": ball_sb[:, 4:6], "bk2": ball_sb[:, 6:8],
                    "bv": ball_sb[:, 8:10], "bu": ball_sb[:, 10:14]}
            wq1_sb = consts.tile([128, CB, K], F8, name="wq1_sb")
            nc.scalar.dma_start(out=wq1_sb, in_=wq1_d.rearrange("(c p) m -> p c m", p=128))
            wq2_sb = consts.tile([128, KB, K], F8, name="wq2_sb")
            nc.sync.dma_start(out=wq2_sb, in_=wq2_d.rearrange("(c p) m -> p c m", p=128))
            wk2_sb = consts.tile([128, KB, K], BF, name="wk2_sb")
            nc.gpsimd.dma_start(out=wk2_sb, in_=wk2_d.rearrange("(c p) m -> p c m", p=128))
            wv_sb = consts.tile([128, CB, K], BF, name="wv_sb")
            nc.gpsimd.dma_start(out=wv_sb, in_=wv_d.rearrange("(c p) m -> p c m", p=128))

            # ---- x chunk prefetch machinery (ramped sizes) ----
            chw = CHMAX * TN
            xch = [None] * nch

            def prefetch(ci):
                if ci >= nch:
                    return
                w = ch_sizes[ci] * TN
                xt = xin.tile([128, CB, chw], F8, tag="x", name="xch")
                off = ch_starts[ci] * TN
                nc.sync.dma_start(out=xt[:, :, :w], in_=x_v[:, :, off:off + w])
                xch[ci] = xt

            prefetch(0)
            prefetch(1)

            wu_sb = consts.tile([128, KB, C], BF, name="wu_sb")
            nc.gpsimd.dma_start(out=wu_sb, in_=wu_d.rearrange("(c p) m -> p c m", p=128))


            ones_f = consts.tile([NCTX, NCTX], F32, name="ones_f")
            nc.vector.memset(ones_f, 1.0)
            ones = consts.tile([NCTX, NCTX], BF, name="ones")
            nc.vector.tensor_copy(ones, ones_f)

            # ---- preamble: k, v projections; Wv = uW' @ v ----
            k1_sb = consts.tile([128, KB, NCTX], BF, name="k1_sb")
            for m in range(KB):
                p = psQ.tile([128, NCTX], F32, tag="mm", name="pk1")
                for c in range(CB):
                    mm(p, wk1_sb[:, c, ts(m, 128)], ct_sb[:, c, :],
                       start=(c == 0), stop=(c == CB - 1))
                nc.scalar.activation(k1_sb[:, m, :], p, AFT.Relu,
                                     bias=bias["bk1"][:, m:m + 1])
            k2_sb = consts.tile([128, KB, NCTX], F8, name="k2_sb")
            for m in range(KB):
                p = psQ.tile([128, NCTX], F32, tag="mm", name="pk2")
                for c in range(KB):
                    mm(p, wk2_sb[:, c, ts(m, 128)], k1_sb[:, c, :],
                       start=(c == 0), stop=(c == KB - 1))
                nc.scalar.activation(k2_sb[:, m, :], p, AFT.Relu,
                                     bias=bias["bk2"][:, m:m + 1])
            v_sb = consts.tile([128, KB, NCTX], BF, name="v_sb")
            for m in range(KB):
                p = psQ.tile([128, NCTX], F32, tag="mm", name="pv")
                for c in range(CB):
                    mm(p, wv_sb[:, c, ts(m, 128)], ct_sb[:, c, :],
                       start=(c == 0), stop=(c == CB - 1))
                nc.scalar.activation(v_sb[:, m, :], p, AFT.Relu,
                                     bias=bias["bv"][:, m:m + 1])
            # WvT [19, C] = v^T @ uW'^T  (lhsT = v_sb blocks, rhs = wu_sb blocks)
            pwv = psS.tile([NCTX, C], F32, tag="s", name="pwv")
            for c in range(KB):
                mm(pwv, v_sb[:, c, :], wu_sb[:, c, :],
                   start=(c == 0), stop=(c == KB - 1))
            wvT_sb = consts.tile([NCTX, C], BF, name="wvT_sb")
            nc.vector.tensor_copy(wvT_sb, pwv)

            # ---- main loop: thin (den/y) matmuls interleaved between fat
            # q-groups so the PE never runs a long thin-MM stretch (HAM),
            # and every engine queue sees its consumers in dependency order.
            state = {}
            ytbuf = {}

            def iteration(i):
                tA = i if i < nt else None
                tP = i - 1 if 1 <= i <= nt else None
                tQ = i - 2 if 2 <= i <= nt + 1 else None
                tB = i - 3 if 3 <= i <= nt + 2 else None

                if tP is not None:
                    stP = state[tP]
                    pden = psS.tile([NCTX, TN], F32, tag="s", name="pden")
                    mm(pden, ones, stP["esim"], start=True, stop=True)

                if tA is not None:
                    ci = chunk_of[tA]
                    if tA == ch_starts[ci] and ci >= 1:
                        prefetch(ci + 1)
                    xoff = (tA - ch_starts[ci]) * TN
                    xt = xch[ci][:, :, xoff:xoff + TN]
                    q1 = work.tile([128, KB, TN], F8, tag="q1", name="q1")
                    q2 = work.tile([128, KB, TN], F8, tag="q2", name="q2")

                if tB is not None:
                    stB = state[tB]
                    if tB % 2 == 0:
                        yt = yout.tile([128, CB, 2 * TN], BF, tag="yt", name="yt")
                        ytbuf[tB // 2] = yt
                    else:
                        yt = ytbuf[tB // 2]
                    yoff = (tB % 2) * TN

                def emit_y(m):
                    if tB is None:
                        return
                    p = psY.tile([128, TN], F32, tag="y", name="py")
                    mm(p, wvT_sb[:, ts(m, 128)], stB["att"], start=True, stop=True)
                    if m < 2:
                        nc.scalar.activation(yt[:, m, yoff:yoff + TN], p, AFT.Relu,
                                             bias=bias["bu"][:, m:m + 1])
                    else:
                        nc.vector.tensor_scalar(yt[:, m, yoff:yoff + TN], p,
                                                bias["bu"][:, m:m + 1], 0.0,
                                                ALU.add, ALU.max)
                    if m == CB - 1 and (tB % 2 == 1 or tB == nt - 1):
                        base = (tB // 2) * 2
                        wdt = (tB - base + 1) * TN
                        nc.gpsimd.dma_start(out=y_v[:, :, base * TN:base * TN + wdt],
                                            in_=yt[:, :, :wdt])
                        ytbuf.pop(tB // 2)

                def emit_q1(m):
                    if tA is None:
                        return
                    p = psQ.tile([128, TN], F32, tag="mm", name="pq1")
                    for c in range(CB):
                        mm(p, wq1_sb[:, c, ts(m, 128)], xt[:, c, :],
                           start=(c == 0), stop=(c == CB - 1))
                    nc.scalar.activation(q1[:, m, :], p, AFT.Relu,
                                         bias=bias["bq1"][:, m:m + 1])

                def emit_q2(m):
                    if tA is None:
                        return
                    p = psQ.tile([128, TN], F32, tag="mm", name="pq2")
                    for c in range(KB):
                        mm(p, wq2_sb[:, c, ts(m, 128)], q1[:, c, :],
                           start=(c == 0), stop=(c == KB - 1))
                    nc.scalar.activation(q2[:, m, :], p, AFT.Relu,
                                         bias=bias["bq2"][:, m:m + 1])

                emit_q1(0)
                emit_y(0)
                emit_q1(1)
                emit_y(1)
                if tQ is not None:
                    stQ = state[tQ]
                    att = work.tile([NCTX, TN], BF, tag="att", name="att")
                    nc.vector.tensor_mul(att, stQ["esim"], stQ["recipb"])
                    stQ["att"] = att
                emit_q2(0)
                emit_y(2)
                emit_q2(1)
                emit_y(3)
                if tP is not None:
                    # recip emitted AFTER the y-relus so the DVE queue is not
                    # head-of-line blocked waiting on the denominator while
                    # y PSUM banks still need draining.
                    recipb = work.tile([NCTX, TN], F32, tag="recipb", name="recipb")
                    nc.vector.reciprocal_approx_fast(recipb, pden)
                    stP["recipb"] = recipb
                if tA is not None:
                    psim = psS.tile([NCTX, TN], F32, tag="s", name="psim")
                    for c in range(KB):
                        mm(psim, k2_sb[:, c, :], q2[:, c, :],
                           start=(c == 0), stop=(c == KB - 1))
                    esim = work.tile([NCTX, TN], BF, tag="esim", name="esim", bufs=3)
                    nc.scalar.activation(esim, psim, AFT.Exp, scale=SOFTMAX_SCALE)
                    state[tA] = {"esim": esim}
                if tB is not None:
                    state.pop(tB)

            for _ in range(repeat):
                for i in range(nt + 3):
                    iteration(i)

    nc.compile()
    return nc


def _prepare_inputs(inputs, npix=NPIX):
    """Fold BN scale into weights, pack biases, shard over batch."""
    f = np.float32

    def fold(bn, conv_b):
        g, be, m, v = [np.asarray(a, dtype=np.float64) for a in bn]
        s = g / np.sqrt(v + EPS)
        t = be - m * s
        bias = np.asarray(conv_b, dtype=np.float64) * s + t
        return s, bias.astype(f)

    def pack(vec):  # [C'] -> [128, C'//128], channel = blk*128 + p
        return np.ascontiguousarray(np.asarray(vec, f).reshape(-1, 128).T)

    def wfold(wname, s, dt):  # [O, I] * s[O] -> transposed [I, O]
        w = np.asarray(inputs[wname], np.float64) * s[:, None]
        return np.ascontiguousarray(w.T.astype(dt))

    sq1, bq1 = fold(inputs["qbn1"], inputs["qb1"])
    sq2, bq2 = fold(inputs["qbn2"], inputs["qb2"])
    sk1, bk1 = fold(inputs["kbn1"], inputs["kb1"])
    sk2, bk2 = fold(inputs["kbn2"], inputs["kb2"])
    sv, bv = fold(inputs["vbn"], inputs["vb"])
    su, bu = fold(inputs["ubn"], inputs["ub"])

    ball = np.concatenate([pack(bq1), pack(bq2), pack(bk1),
                           pack(bk2), pack(bv), pack(bu)], axis=1)
    base = {
        "wq1": wfold("qW1", sq1, F8_NP), "wq2": wfold("qW2", sq2, F8_NP),
        "wk1": wfold("kW1", sk1, BF_NP), "wk2": wfold("kW2", sk2, BF_NP),
        "wv": wfold("vW", sv, BF_NP), "wu": wfold("uW", su, BF_NP),
        "ball": np.ascontiguousarray(ball),
    }
    x = np.asarray(inputs["x"], f)
    ctx = np.asarray(inputs["context"], f)
    in_maps = []
    for b_i in range(x.shape[0]):
        m = dict(base)
        m["x"] = np.ascontiguousarray(
            x[b_i].reshape(C, -1)[:, :npix].astype(F8_NP))
        m["ctxt"] = np.ascontiguousarray(
            ctx[b_i].reshape(C, NCTX).astype(BF_NP))
        in_maps.append(m)
    return in_maps


_NC_CACHE = {}


def _get_nc(npix=NPIX):
    key = (npix, TN)
    if key not in _NC_CACHE:
        _NC_CACHE[key] = _build(npix)
    return _NC_CACHE[key]


def run(inputs, trace=False, **kwargs):
    """Run on 8 cores; returns (y [8,512,128,128], BassKernelResults)."""
    nc = _get_nc()
    in_maps = _prepare_inputs(inputs)
    res = bass_utils.run_bass_kernel_spmd(
        nc, in_maps, core_ids=list(range(B)), trace=trace, **kwargs)
    y = np.stack([np.asarray(res.results[b]["y"], np.float32).reshape(C, H, W)
                  for b in range(B)])
    return y.astype(np.float32), res


def kernel(**inputs):
    y, _ = run(inputs)
    return y


# revision 25
# speedup vs baseline: 1.0123x; 1.0123x over previous
"""Trainium2 Bass kernel for nn_ContextRelation_Module (dense_transformer).

Data-parallel over batch: 8 batches -> 8 NeuronCores, one batch each.

Per-core program (B=1 slice); q-branch in fp8e4m3, attention/y in bf16,
PSUM fp32:
  x     [512, 16384]  (C_in, H*W)  fp8
  q1    = relu(W_q1' @ x + b1)            [256, 16384]   (BN scale folded into W)
  q2    = relu(W_q2' @ q1 + b2)           [256, 16384]
  k,v   = relu-stacks of the 19-token context  (preamble)
  Wv    = W_u' @ v                         [512, 19]      (preamble;  y-GEMM collapse:
                                            y = relu(W_u'@(v@attT) + bu) = relu(Wv@attT + bu))
  simT  = k^T @ q2 * (1/16)                [19, 16384]
  esim  = exp(simT)
  den   = ones[19,19]^T @ esim             [19, 16384]   (denominator, pre-broadcast)
  att   = esim * approx_recip(den)         [19, 16384]
  y     = relu(Wv^T.T @ att + bu)          [512, 16384]

Key design points:
  * y = (uW@v) @ attT: the [K,HW] ctx tensor and its 8 matmuls + 2 PSUM
    drains per tile collapse into 4 contraction-19 matmuls (26 -> 19
    N=512 matmuls per tile on the PE).
  * fp8 q-branch: the softmax logits are tiny (sim*1/16), so attention
    output is insensitive to q-path quantization (measured 2.9e-3
    absmax-rel, same as all-bf16).  x HBM traffic drops to 8 MB.
  * BN scale folded into weights host-side; every BN-ReLU is a single
    activation/tensor_scalar op.
  * softmax denominator broadcast merged into the ones-matmul
    (ones[19,19]); reciprocal via the ~5x faster approx custom DVE op.
  * pointwise split: Scalar (q relus, exp, 2 y-relus), Vector (recip,
    att mul, 2 y-relus), GpSimd (y DMA).
  * x DMAed in ramped chunks (1,1,2,8,... tiles) so the first q1 matmul
    is not gated on a large transfer; constant DMA issues are spread
    across the sync/scalar/gpsimd queues (each software-DGE dma_start
    costs ~0.7us of issuing-engine time).

The per-512-column tile pipeline is software-pipelined across 4 stages
(den | y | att | x-dma+projections+sim+exp); thin matmuls (den, y) are
interleaved between the fat q-groups, and the DVE recip is emitted after
the y-relus so no engine queue head-of-line blocks on a fresh value.
Measured on HW (NTFF profile): ~202 us device span per launch
(vs 316 us for the v1 fp32r kernel), PE-bound at ~300 ns per
LDWEIGHTS+MATMUL pair.  DoubleRow fp8 was tried and measured SLOWER
(~750 ns per K=256 matmul - the walrus codegen path streams the two
interleaved planes sequentially); fp8 here buys HBM traffic and weight-
load time, not matmul rate.
"""

import numpy as np

import concourse.bacc as bacc
import concourse.bass as bass
import concourse.mybir as mybir
import concourse.tile as tile
from concourse import bass_utils
from concourse.bass import ts
from concourse import bass_isa

AFT = mybir.ActivationFunctionType
ALU = mybir.AluOpType
F32 = mybir.dt.float32
BF = mybir.dt.bfloat16
F8 = mybir.dt.float8e4
BF_NP = mybir.dt.np(mybir.dt.bfloat16)
F8_NP = mybir.dt.np(mybir.dt.float8e4)

# problem dims (hardcoded per contract)
B = 8
C = 512            # input/output channels
K = 256            # key_channels
H = 128
W = 128
NCTX = 19          # context tokens
NPIX = H * W       # 16384 pixels per batch
CB = C // 128      # 4 partition blocks of C
KB = K // 128      # 2 partition blocks of K
EPS = 1e-5
SOFTMAX_SCALE = K ** -0.5   # 1/16

# tunables
TN = 512           # free-dim tile (one PSUM bank of fp32)
CHMAX = 8          # steady-state pixel tiles per x-DMA chunk


def _chunk_plan(nt):
    """Ramped chunk sizes so tile 0 is gated on a 1-tile transfer."""
    sizes = []
    for s in (1, 1, 2):
        if sum(sizes) >= nt:
            break
        sizes.append(min(s, nt - sum(sizes)))
    while sum(sizes) < nt:
        sizes.append(min(CHMAX, nt - sum(sizes)))
    starts = [sum(sizes[:i]) for i in range(len(sizes))]
    return sizes, starts


def _build(npix=NPIX, repeat=1):
    """Build + compile the per-core Bass module.

    repeat>1 re-runs the whole pixel loop (same input/output) for
    differential timing: t(R) - t(1) = (R-1) * T_kernel.
    """
    nt = npix // TN
    ch_sizes, ch_starts = _chunk_plan(nt)
    nch = len(ch_sizes)
    chunk_of = []
    for i, s in enumerate(ch_sizes):
        chunk_of += [i] * s
    nc = bacc.Bacc("TRN2", target_bir_lowering=False, debug=False)

    x_d = nc.dram_tensor("x", [C, npix], F8, kind="ExternalInput").ap()
    ct_d = nc.dram_tensor("ctxt", [C, NCTX], BF, kind="ExternalInput").ap()
    wq1_d = nc.dram_tensor("wq1", [C, K], F8, kind="ExternalInput").ap()
    wq2_d = nc.dram_tensor("wq2", [K, K], F8, kind="ExternalInput").ap()
    wk1_d = nc.dram_tensor("wk1", [C, K], BF, kind="ExternalInput").ap()
    wk2_d = nc.dram_tensor("wk2", [K, K], BF, kind="ExternalInput").ap()
    wv_d = nc.dram_tensor("wv", [C, K], BF, kind="ExternalInput").ap()
    wu_d = nc.dram_tensor("wu", [K, C], BF, kind="ExternalInput").ap()
    # biases packed into one tensor: [bq1|bq2|bk1|bk2|bv](2 each) + bu(4)
    ball_d = nc.dram_tensor("ball", [128, 14], F32, kind="ExternalInput").ap()
    y_d = nc.dram_tensor("y", [C, npix], BF, kind="ExternalOutput").ap()

    x_v = x_d.rearrange("(c p) n -> p c n", p=128)
    y_v = y_d.rearrange("(c p) n -> p c n", p=128)

    mm = nc.tensor.matmul

    with tile.TileContext(nc) as tc, nc.allow_low_precision(reason="fp8/bf16 operands"):
        with (
            tc.tile_pool(name="consts", bufs=1) as consts,
            tc.tile_pool(name="xin", bufs=3) as xin,
            tc.tile_pool(name="yout", bufs=2) as yout,
            tc.tile_pool(name="work", bufs=2) as work,
            tc.tile_pool(name="psQ", bufs=4, space="PSUM") as psQ,
            tc.tile_pool(name="psS", bufs=2, space="PSUM") as psS,
            tc.tile_pool(name="psY", bufs=2, space="PSUM") as psY,
        ):
            # ---- constants; DMA issues spread across engine queues so the
            # ~0.7us-per-issue descriptor cost does not serialize startup.
            # Tile-0's inputs (chunk 0, wq1, biases) are issued FIRST so its
            # q-matmuls (emitted ahead of the k/v preamble) start ~3us sooner.
            chw = CHMAX * TN
            xch = [None] * nch

            def prefetch(ci):
                if ci >= nch:
                    return
                w = ch_sizes[ci] * TN
                xt = xin.tile([128, CB, chw], F8, tag="x", name="xch")
                off = ch_starts[ci] * TN
                nc.sync.dma_start(out=xt[:, :, :w], in_=x_v[:, :, off:off + w])
                xch[ci] = xt

            prefetch(0)
            wq1_sb = consts.tile([128, CB, K], F8, name="wq1_sb")
            nc.scalar.dma_start(out=wq1_sb, in_=wq1_d.rearrange("(c p) m -> p c m", p=128))
            ball_sb = consts.tile([128, 14], F32, name="ball_sb")
            nc.gpsimd.dma_start(out=ball_sb, in_=ball_d)
            bias = {"bq1": ball_sb[:, 0:2], "bq2": ball_sb[:, 2:4],
                    "bk1": ball_sb[:, 4:6], "bk2": ball_sb[:, 6:8],
                    "bv": ball_sb[:, 8:10], "bu": ball_sb[:, 10:14]}
            wq2_sb = consts.tile([128, KB, K], F8, name="wq2_sb")
            nc.scalar.dma_start(out=wq2_sb, in_=wq2_d.rearrange("(c p) m -> p c m", p=128))
            wk1_sb = consts.tile([128, CB, K], BF, name="wk1_sb")
            nc.sync.dma_start(out=wk1_sb, in_=wk1_d.rearrange("(c p) m -> p c m", p=128))
            ct_sb = consts.tile([128, CB, NCTX], BF, name="ct_sb")
            nc.gpsimd.dma_start(out=ct_sb, in_=ct_d.rearrange("(c p) m -> p c m", p=128))
            prefetch(1)
            wk2_sb = consts.tile([128, KB, K], BF, name="wk2_sb")
            nc.gpsimd.dma_start(out=wk2_sb, in_=wk2_d.rearrange("(c p) m -> p c m", p=128))
            wv_sb = consts.tile([128, CB, K], BF, name="wv_sb")
            nc.gpsimd.dma_start(out=wv_sb, in_=wv_d.rearrange("(c p) m -> p c m", p=128))
            wu_sb = consts.tile([128, KB, C], BF, name="wu_sb")
            nc.gpsimd.dma_start(out=wu_sb, in_=wu_d.rearrange("(c p) m -> p c m", p=128))


            ones_f = consts.tile([NCTX, NCTX], F32, name="ones_f")
            nc.vector.memset(ones_f, 1.0)
            ones = consts.tile([NCTX, NCTX], BF, name="ones")
            nc.vector.tensor_copy(ones, ones_f)

            # ---- tile 0's q-projections, ahead of the k/v preamble so the
            # PE starts on them as soon as chunk0+wq1 land (the preamble's
            # wk1/ct arrive later and would otherwise block the PE queue) ----
            t0_state = {}
            xt0 = None
            if nt > 0:
                xt0 = xch[0][:, :, 0:TN]
                q1_0 = work.tile([128, KB, TN], F8, tag="q1", name="q1")
                for m_ in range(KB):
                    p_ = psQ.tile([128, TN], F32, tag="mm", name="pq1")
                    for c_ in range(CB):
                        mm(p_, wq1_sb[:, c_, ts(m_, 128)], xt0[:, c_, :],
                           start=(c_ == 0), stop=(c_ == CB - 1))
                    nc.scalar.activation(q1_0[:, m_, :], p_, AFT.Relu,
                                         bias=bias["bq1"][:, m_:m_ + 1])
                q2_0 = work.tile([128, KB, TN], F8, tag="q2", name="q2")
                for m_ in range(KB):
                    p_ = psQ.tile([128, TN], F32, tag="mm", name="pq2")
                    for c_ in range(KB):
                        mm(p_, wq2_sb[:, c_, ts(m_, 128)], q1_0[:, c_, :],
                           start=(c_ == 0), stop=(c_ == KB - 1))
                    nc.scalar.activation(q2_0[:, m_, :], p_, AFT.Relu,
                                         bias=bias["bq2"][:, m_:m_ + 1])
                t0_state["q2"] = q2_0

            # ---- preamble: k, v projections; Wv = uW' @ v ----
            k1_sb = consts.tile([128, KB, NCTX], BF, name="k1_sb")
            for m in range(KB):
                p = psQ.tile([128, NCTX], F32, tag="mm", name="pk1")
                for c in range(CB):
                    mm(p, wk1_sb[:, c, ts(m, 128)], ct_sb[:, c, :],
                       start=(c == 0), stop=(c == CB - 1))
                nc.scalar.activation(k1_sb[:, m, :], p, AFT.Relu,
                                     bias=bias["bk1"][:, m:m + 1])
            k2_sb = consts.tile([128, KB, NCTX], F8, name="k2_sb")
            for m in range(KB):
                p = psQ.tile([128, NCTX], F32, tag="mm", name="pk2")
                for c in range(KB):
                    mm(p, wk2_sb[:, c, ts(m, 128)], k1_sb[:, c, :],
                       start=(c == 0), stop=(c == KB - 1))
                nc.scalar.activation(k2_sb[:, m, :], p, AFT.Relu,
                                     bias=bias["bk2"][:, m:m + 1])
            v_sb = consts.tile([128, KB, NCTX], BF, name="v_sb")
            for m in range(KB):
                p = psQ.tile([128, NCTX], F32, tag="mm", name="pv")
                for c in range(CB):
                    mm(p, wv_sb[:, c, ts(m, 128)], ct_sb[:, c, :],
                       start=(c == 0), stop=(c == CB - 1))
                nc.scalar.activation(v_sb[:, m, :], p, AFT.Relu,
                                     bias=bias["bv"][:, m:m + 1])
            # WvT [19, C] = v^T @ uW'^T  (lhsT = v_sb blocks, rhs = wu_sb blocks)
            pwv = psS.tile([NCTX, C], F32, tag="s", name="pwv")
            for c in range(KB):
                mm(pwv, v_sb[:, c, :], wu_sb[:, c, :],
                   start=(c == 0), stop=(c == KB - 1))
            wvT_sb = consts.tile([NCTX, C], BF, name="wvT_sb")
            nc.vector.tensor_copy(wvT_sb, pwv)


            # ---- main loop: thin (den/y) matmuls interleaved between fat
            # q-groups so the PE never runs a long thin-MM stretch (HAM),
            # and every engine queue sees its consumers in dependency order.
            state = {}
            ytbuf = {}

            def iteration(i):
                tA = i if i < nt else None
                tP = i - 1 if 1 <= i <= nt else None
                tQ = i - 2 if 2 <= i <= nt + 1 else None
                tB = i - 3 if 3 <= i <= nt + 2 else None

                if tP is not None:
                    stP = state[tP]
                    pden = psS.tile([NCTX, TN], F32, tag="s", name="pden")
                    mm(pden, ones, stP["esim"], start=True, stop=True)

                tA_q = tA if (tA is not None and tA > 0) else None
                if tA_q is not None:
                    ci = chunk_of[tA_q]
                    if tA_q == ch_starts[ci] and ci >= 1:
                        prefetch(ci + 1)
                    xoff = (tA_q - ch_starts[ci]) * TN
                    xt = xch[ci][:, :, xoff:xoff + TN]
                    q1 = work.tile([128, KB, TN], F8, tag="q1", name="q1")
                    q2 = work.tile([128, KB, TN], F8, tag="q2", name="q2")

                if tB is not None:
                    stB = state[tB]
                    if tB % 2 == 0:
                        yt = yout.tile([128, CB, 2 * TN], BF, tag="yt", name="yt")
                        ytbuf[tB // 2] = yt
                    else:
                        yt = ytbuf[tB // 2]
                    yoff = (tB % 2) * TN

                def emit_y(m):
                    if tB is None:
                        return
                    p = psY.tile([128, TN], F32, tag="y", name="py")
                    mm(p, wvT_sb[:, ts(m, 128)], stB["att"], start=True, stop=True)
                    if m < 2:
                        nc.scalar.activation(yt[:, m, yoff:yoff + TN], p, AFT.Relu,
                                             bias=bias["bu"][:, m:m + 1])
                    else:
                        nc.vector.tensor_scalar(yt[:, m, yoff:yoff + TN], p,
                                                bias["bu"][:, m:m + 1], 0.0,
                                                ALU.add, ALU.max)
                    if m == CB - 1:
                        base = (tB // 2) * 2
                        if tB % 2 == 0 and tB == nt - 2:
                            # early-flush the penultimate tile so the final
                            # transfer is one tile, shortening the drain
                            nc.gpsimd.dma_start(
                                out=y_v[:, :, base * TN:(base + 1) * TN],
                                in_=yt[:, :, :TN])
                        elif tB % 2 == 1:
                            if tB == nt - 1 and nt >= 2:
                                nc.gpsimd.dma_start(
                                    out=y_v[:, :, tB * TN:(tB + 1) * TN],
                                    in_=yt[:, :, TN:2 * TN])
                            else:
                                nc.gpsimd.dma_start(
                                    out=y_v[:, :, base * TN:(base + 2) * TN],
                                    in_=yt)
                            ytbuf.pop(tB // 2)
                        elif tB == nt - 1:
                            nc.gpsimd.dma_start(
                                out=y_v[:, :, base * TN:(base + 1) * TN],
                                in_=yt[:, :, :TN])
                            ytbuf.pop(tB // 2)

                def emit_q1(m):
                    if tA_q is None:
                        return
                    p = psQ.tile([128, TN], F32, tag="mm", name="pq1")
                    for c in range(CB):
                        mm(p, wq1_sb[:, c, ts(m, 128)], xt[:, c, :],
                           start=(c == 0), stop=(c == CB - 1))
                    nc.scalar.activation(q1[:, m, :], p, AFT.Relu,
                                         bias=bias["bq1"][:, m:m + 1])

                def emit_q2(m):
                    if tA_q is None:
                        return
                    p = psQ.tile([128, TN], F32, tag="mm", name="pq2")
                    for c in range(KB):
                        mm(p, wq2_sb[:, c, ts(m, 128)], q1[:, c, :],
                           start=(c == 0), stop=(c == KB - 1))
                    nc.scalar.activation(q2[:, m, :], p, AFT.Relu,
                                         bias=bias["bq2"][:, m:m + 1])

                emit_q1(0)
                emit_y(0)
                emit_q1(1)
                emit_y(1)
                if tQ is not None:
                    stQ = state[tQ]
                    att = work.tile([NCTX, TN], BF, tag="att", name="att")
                    nc.vector.tensor_mul(att, stQ["esim"], stQ["recipb"])
                    stQ["att"] = att
                emit_q2(0)
                emit_y(2)
                emit_q2(1)
                emit_y(3)
                if tP is not None:
                    # recip emitted AFTER the y-relus so the DVE queue is not
                    # head-of-line blocked waiting on the denominator while
                    # y PSUM banks still need draining.
                    recipb = work.tile([NCTX, TN], F32, tag="recipb", name="recipb")
                    nc.vector.reciprocal_approx_fast(recipb, pden)
                    stP["recipb"] = recipb
                if tA is not None:
                    q2s = t0_state["q2"] if tA == 0 else q2
                    psim = psS.tile([NCTX, TN], F32, tag="s", name="psim")
                    for c in range(KB):
                        mm(psim, k2_sb[:, c, :], q2s[:, c, :],
                           start=(c == 0), stop=(c == KB - 1))
                    esim = work.tile([NCTX, TN], BF, tag="esim", name="esim", bufs=3)
                    nc.scalar.activation(esim, psim, AFT.Exp, scale=SOFTMAX_SCALE)
                    state[tA] = {"esim": esim}
                if tB is not None:
                    state.pop(tB)

            for _ in range(repeat):
                for i in range(nt + 3):
                    iteration(i)

    nc.compile()
    return nc


def _prepare_inputs(inputs, npix=NPIX):
    """Fold BN scale into weights, pack biases, shard over batch."""
    f = np.float32

    def fold(bn, conv_b):
        g, be, m, v = [np.asarray(a, dtype=np.float64) for a in bn]
        s = g / np.sqrt(v + EPS)
        t = be - m * s
        bias = np.asarray(conv_b, dtype=np.float64) * s + t
        return s, bias.astype(f)

    def pack(vec):  # [C'] -> [128, C'//128], channel = blk*128 + p
        return np.ascontiguousarray(np.asarray(vec, f).reshape(-1, 128).T)

    def wfold(wname, s, dt):  # [O, I] * s[O] -> transposed [I, O]
        w = np.asarray(inputs[wname], np.float64) * s[:, None]
        return np.ascontiguousarray(w.T.astype(dt))

    sq1, bq1 = fold(inputs["qbn1"], inputs["qb1"])
    sq2, bq2 = fold(inputs["qbn2"], inputs["qb2"])
    sk1, bk1 = fold(inputs["kbn1"], inputs["kb1"])
    sk2, bk2 = fold(inputs["kbn2"], inputs["kb2"])
    sv, bv = fold(inputs["vbn"], inputs["vb"])
    su, bu = fold(inputs["ubn"], inputs["ub"])

    ball = np.concatenate([pack(bq1), pack(bq2), pack(bk1),
                           pack(bk2), pack(bv), pack(bu)], axis=1)
    base = {
        "wq1": wfold("qW1", sq1, F8_NP), "wq2": wfold("qW2", sq2, F8_NP),
        "wk1": wfold("kW1", sk1, BF_NP), "wk2": wfold("kW2", sk2, BF_NP),
        "wv": wfold("vW", sv, BF_NP), "wu": wfold("uW", su, BF_NP),
        "ball": np.ascontiguousarray(ball),
    }
    x = np.asarray(inputs["x"], f)
    ctx = np.asarray(inputs["context"], f)
    in_maps = []
    for b_i in range(x.shape[0]):
        m = dict(base)
        m["x"] = np.ascontiguousarray(
            x[b_i].reshape(C, -1)[:, :npix].astype(F8_NP))
        m["ctxt"] = np.ascontiguousarray(
            ctx[b_i].reshape(C, NCTX).astype(BF_NP))
        in_maps.append(m)
    return in_maps


_NC_CACHE = {}


def _get_nc(npix=NPIX):
    key = (npix, TN)
    if key not in _NC_CACHE:
        _NC_CACHE[key] = _build(npix)
    return _NC_CACHE[key]


def run(inputs, trace=False, **kwargs):
    """Run on 8 cores; returns (y [8,512,128,128], BassKernelResults)."""
    nc = _get_nc()
    in_maps = _prepare_inputs(inputs)
    res = bass_utils.run_bass_kernel_spmd(
        nc, in_maps, core_ids=list(range(B)), trace=trace, **kwargs)
    y = np.stack([np.asarray(res.results[b]["y"], np.float32).reshape(C, H, W)
                  for b in range(B)])
    return y.astype(np.float32), res


def kernel(**inputs):
    y, _ = run(inputs)
    return y
